# revision 18
# baseline (speedup 1.0000x reference)
"""Trainium2 Bass kernel for nn_DigitConvolutionalModel (dense_cnn).

Model: y = relu(conv3x3(x) @ w1.T + b1) @ w2.T + b2, x: [65536, 784] f32.

Strategy (v8):
  * Conv3x3 and FC1 fuse on the host into one effective weight
    W1e = w1 @ C with shape [128, 784] (C is the sparse conv operator),
    so the device runs a pure GEMM pipeline:
    y = relu(x @ W1e.T + b1) @ w2.T + b2.
  * Pure data parallel over 8 NeuronCores: each core gets 8192 rows of x.
    No collectives; each core produces its own output shard.
  * x travels as fp8e3 (e3m4), scaled by 2 on the host (absmax 10.8 of
    15.5) with the inverse folded into W1e, which stays fp16 — the PE
    accepts mixed operand dtypes (HW-verified), so weight quantization
    adds no error and x quantization alone costs ~1.3e-2 rel_fro
    (gate: 2e-2). Quarter the HBM traffic of f32 for x; 1 cycle/row on
    the PE; all accumulation stays fp32 in PSUM. DMA delivers each
    512-column block in ~1.1 us vs ~1.7 us of PE work, so the PE never
    starves mid-stream (no HAM clock-gate oscillation).
  * x streams on the SP HWDGE ring only (the ACT ring must stay clear
    for the per-block bias-adds) in a tapered schedule — 6x512,
    4x1024, 2x512 columns — every load one contiguous region
    (128 descriptors) via host pre-tiling, every load with its OWN SBUF
    slot (x fits in SBUF) so nothing waits on buffer recycling. Fine
    granularity up front keeps the PE fed while the stream ramps.
  * Weights / biases load on the ACT HWDGE ring in parallel. The
    16-feature contraction tail (features 768:784, whole batch) is
    packed [128, 2048] across 4 row-groups of 32 partitions so its DMA
    uses all 16 SDMA engines; w1e's tail rows are replicated at
    partition offsets 0/32/64/96 so each block's tail matmul reads its
    group via tile_position.
  * Per 512-column block: 6 accumulating FC1 matmuls + 1 tail matmul
    into a PSUM bank (4-bank rotation), fused bias+ReLU on the vector
    engine (PSUM -> SBUF fp16). The [10, 512] FC2 matmul runs
    SOFTWARE-PIPELINED one block behind FC1 so the PE never waits on
    the relu inside its own stream; its output lands at PSUM partition
    offset 32*(bi%4) (col-group tile_position) so the fp16 output
    accumulator is [128, 2048] — the final store then rides all 16
    SDMA engines (a [10, 8192] layout would ride 2 and add ~6 us).
  * FC2 bias on the scalar engine; ONE final store from the ACT
    sequencer (program order after the last act: no waits).
  * Cross-engine waits are absorbed into the PE stream with tiny dummy
    bf16 ldweights "probes"; the few remaining multi-waits are split via
    event semaphores (bass_rust.generate_event_semaphores).
  * Nine dummy matmuls over a zeroed scratch tile during the DMA-bound
    startup window pre-warm the PE's HAM clock gate to 2.4 GHz.
"""

import os

import ml_dtypes
import numpy as np

import concourse.bass as bass
import concourse.mybir as mybir
import concourse.tile as tile
from concourse.bass import ts
from concourse.bass_utils import run_bass_kernel_spmd

H = W = 28
KH = KW = 3
CIN = H * W  # 784
HID = 128
OUT = 10
B_TOTAL = 65536
NCORES = 8
BS = B_TOTAL // NCORES  # 8192 rows per core
NB = 512  # batch columns per block (fp32 PSUM bank limit)
NBLK = BS // NB  # 16
KCH = 128
KC = 6  # full chunks (6 * 128 = 768)
KTAIL = CIN - KC * KCH  # 16
NGRP = 4  # tail row-groups / FC2 col-groups (32 partitions each)
TGC = BS // NGRP  # tail columns per group (2048)
# tapered x load schedule (columns per load): fine up front
SCHED = [NB] * 6 + [2 * NB] * 4 + [NB] * 2

MM_MODE = os.environ.get("BASS_MM_DT", "f8")
HOST_DT = np.float16
X_DT = ml_dtypes.float8_e3m4
X_SCALE = 2.0  # folded into W1e on the host


def _build_nc():
    f32 = mybir.dt.float32
    mdt = mybir.dt.float16
    xdt = mybir.dt.float8e3
    nc = bass.Bass()
    # x, host-pretiled per load: xa/xm entries are each one contiguous
    # [128, 6, ncols] region (features 0:768)
    xa = nc.dram_tensor("xa", [8, KCH, KC, NB], xdt, kind="ExternalInput")
    xm = nc.dram_tensor("xm", [4, KCH, KC, 2 * NB], xdt, kind="ExternalInput")
    # x contraction tail (features 768:784) packed into 4 row-groups:
    # partition 32g+j = tail feature j of blocks 4g..4g+3
    xtl = nc.dram_tensor("xtl", [KCH, TGC], xdt, kind="ExternalInput")
    # all fp16 weights packed into one tensor -> one DMA:
    # cols 0:768 = w1e chunks [k, c, m]; rows 32g:32g+16 of cols 768:896
    # = the 16-row w1e tail (replicated per row-group g); cols 896:906 =
    # w2t
    wpk = nc.dram_tensor("wpk", [KCH, 906], mdt, kind="ExternalInput")
    # biases in one f32 tensor: col 0 = b1, col 1 rows 32j:32j+10 = b2
    # (replicated per FC2 col-group j)
    bd = nc.dram_tensor("bd", [HID, 2], f32, kind="ExternalInput")
    # output, fp16, col-group packed: yt[32*(bi%4)+r, (bi//4)*512+c] =
    # y[bi*512+c, r]
    yt = nc.dram_tensor("yt", [KCH, NGRP * NB], mdt, kind="ExternalOutput")

    with tile.TileContext(nc) as tc:
        with (
            tc.tile_pool(name="consts", bufs=1) as consts,
            tc.tile_pool(name="xin", bufs=1) as xin,
            tc.tile_pool(name="hpool", bufs=NBLK) as hpool,
            tc.tile_pool(name="ps1", bufs=5, space="PSUM") as ps1p,
            tc.tile_pool(name="ps2", bufs=3, space="PSUM") as ps2p,
        ):
            # All x loads on the SP ring, finest blocks first.
            loads = []
            x_tl = None
            for li, ncols in enumerate(SCHED):
                x_t = xin.tile(
                    [KCH, KC, ncols], xdt, tag=f"x{li}", bufs=1, name=f"x_{li}"
                )
                if li < 6:
                    srcap = xa[li][:]
                elif li < 10:
                    srcap = xm[li - 6][:]
                else:
                    srcap = xa[li - 4][:]
                nc.sync.dma_start(x_t[:], srcap)
                loads.append(x_t)
                if li == 1:
                    # tail after the first two loads: block 0 needs it
                    # only at its 7th matmul (~1.5 us later)
                    x_tl = consts.tile([KCH, TGC], xdt)
                    nc.sync.dma_start(x_tl[:], xtl[:])

            # block bi -> (x tile, column offset)
            def block_src(bi):
                if bi < 6:
                    return loads[bi], 0
                if bi < 14:
                    return loads[6 + (bi - 6) // 2], ((bi - 6) % 2) * NB
                return loads[10 + bi - 14], 0

            # Weights / biases on the ACT ring, in parallel.
            wpk_t = consts.tile([KCH, 906], mdt)
            nc.scalar.dma_start(wpk_t[:], wpk[:])
            w1_t = wpk_t[:, 0:768].rearrange("k (c m) -> k c m", c=KC)
            w2_t = wpk_t[:, 896:906]
            bd_t = consts.tile([HID, 2], f32)
            nc.scalar.dma_start(bd_t[:], bd[:])
            b1_t = bd_t[:, 0:1]

            # fp16 output accumulator [128, 2048]; one store at the end.
            o_all = consts.tile([KCH, NGRP * NB], mdt)

            # Tiny dummy bf16 ldweights "probes" absorb cross-engine
            # waits into the PE's in-order stream ahead of each matmul
            # group (walrus: one sync wait per instruction; the loaded
            # garbage weight is irrelevant, real matmuls self-load).
            def probe(ap, cast=True):
                ap = ap[0:1, 0:1]
                if cast:
                    ap = ap.bitcast(mybir.dt.bfloat16)
                nc.tensor.ldweights(ap)

            # HAM warm-up FIRST: ~8 cold dummy matmuls fill the PE's
            # ~3.4 us activity window back-to-back (no waits), so the
            # clock gate is at 2.4 GHz before the first real matmul.
            # The memset is the first DVE op (ahead of b1_probe, whose
            # wait on the bd DMA would delay it).
            scratch = consts.tile([KCH, NB], mdt)
            nc.vector.memset(scratch[:], 0.0)
            psd = ps2p.tile([HID, NB], f32, tag="warm", bufs=1)

            def dummy_mm():
                nc.tensor.matmul(
                    psd[:], scratch[:, 0:HID], scratch[:], start=True, stop=True
                )

            for _ in range(8):
                dummy_mm()

            # Pre-touch the bias tiles on their consumer engines (b1 on
            # DVE, b2 on ACT) so relu / bias-add need no extra wait.
            b1_probe = consts.tile([1, 1], f32)
            nc.vector.tensor_copy(b1_probe[:], b1_t[0:1, 0:1])
            b2_probe = consts.tile([1, 1], f32)
            nc.scalar.copy(b2_probe[:], bd_t[0:1, 1:2])

            probe(w1_t[:, 0, :])
            probe(x_tl[:], cast=False)
            probe(w2_t[:])

            hs = [None] * NBLK

            def fc2_quad(q):
                """FC2 for quad q (software-pipelined one quad late).
                A tiny N=1 matmul opens the accumulation group (clears
                has_written + pending-zero for the bank); the 4 FC2
                matmuls then target col-groups j=0..3 of that one bank
                (disjoint partitions, start=False = overwrite-where-
                unwritten) and run concurrently in the array; one
                [128, 512] bias-add evacuates the whole quad."""
                for j in range(NGRP):
                    probe(hs[NGRP * q + j][:])
                ps2 = ps2p.tile([KCH, NB], f32, tag="ps2", bufs=2)
                nc.tensor.matmul(
                    ps2[:, 0:1],
                    scratch[:, 0:HID],
                    scratch[:, 0:1],
                    start=True,
                    stop=False,
                )
                for j in range(NGRP):
                    nc.tensor.matmul(
                        ps2[32 * j : 32 * j + OUT, :],
                        w2_t[:],
                        hs[NGRP * q + j][:],
                        start=False,
                        stop=(j == NGRP - 1),
                        tile_position=(0, 32 * j),
                    )
                nc.scalar.activation(
                    o_all[:, ts(q, NB)],
                    ps2[:],
                    mybir.ActivationFunctionType.Identity,
                    bias=bd_t[:, 1:2],
                )

            for q in range(NGRP):
                pss = []
                # 24 main FC1 matmuls for blocks 4q..4q+3
                for j in range(NGRP):
                    bi = NGRP * q + j
                    x_t, off = block_src(bi)
                    probe(x_t[:, 0, off : off + 1], cast=False)
                    ps = ps1p.tile([HID, NB], f32, tag="ps")
                    for c in range(KC):
                        nc.tensor.matmul(
                            ps[:],
                            w1_t[:, c, :],
                            x_t[:, c, off : off + NB],
                            start=(c == 0),
                            stop=False,
                        )
                    pss.append(ps)
                # 4 tail matmuls on distinct row-groups: concurrent
                for j in range(NGRP):
                    nc.tensor.matmul(
                        pss[j][:],
                        wpk_t[32 * j : 32 * j + KTAIL, 768:896],
                        x_tl[32 * j : 32 * j + KTAIL, ts(q, NB)],
                        start=False,
                        stop=True,
                        tile_position=(32 * j, 0),
                    )
                # relu+bias on DVE: h = max(ps + b1, 0), fp16 out
                for j in range(NGRP):
                    bi = NGRP * q + j
                    h = hpool.tile([HID, NB], mdt, tag="h", name=f"h_{bi}")
                    nc.vector.tensor_scalar(
                        h[:],
                        pss[j][:],
                        b1_t[:],
                        0.0,
                        mybir.AluOpType.add,
                        mybir.AluOpType.max,
                    )
                    hs[bi] = h
                if q >= 1:
                    fc2_quad(q - 1)
                else:
                    dummy_mm()
                    dummy_mm()
            # bulk of the store overlaps the last quad's FC2 work
            nc.scalar.dma_start(
                yt[:, 0 : (NGRP - 1) * NB], o_all[:, 0 : (NGRP - 1) * NB]
            )
            fc2_quad(NGRP - 1)

            # Stores from the ACT sequencer: program order after the acts
            # means they need no cross-engine waits at all.
            nc.scalar.dma_start(
                yt[:, (NGRP - 1) * NB :], o_all[:, (NGRP - 1) * NB :]
            )

    # This walrus build allows one sync-wait per instruction; Tile emits
    # multi-waits in a few places. Split them into event-semaphore
    # chains, same as bacc.compile() does.
    import bass_rust

    bass_rust.generate_event_semaphores(nc)
    return nc


def _fuse_conv_fc1(conv_w, w1):
    """W1e = w1 @ C where C is the 3x3 valid-conv operator [676, 784]."""
    cw = np.asarray(conv_w, np.float64).reshape(KH, KW)
    w1_r = np.asarray(w1, np.float64).reshape(HID, H - KH + 1, W - KW + 1)
    w1e = np.zeros((HID, H, W), np.float64)
    for a in range(KH):
        for b in range(KW):
            w1e[:, a : a + H - KH + 1, b : b + W - KW + 1] += w1_r * cw[a, b]
    return w1e.reshape(HID, CIN).astype(np.float32)


def _tile_cols(x_shard, cs, ncols):
    """[128, 6, ncols] contiguous device layout for columns cs:cs+ncols."""
    return (
        x_shard[cs : cs + ncols, : KC * KCH]
        .reshape(ncols, KC, KCH)
        .transpose(2, 1, 0)
        .astype(X_DT)
    )


def _core_x(x_shard):
    """Pre-tile one core's x rows [BS, 784] into the device layout.
    x arrives pre-scaled by X_SCALE."""
    xa = np.stack(
        [_tile_cols(x_shard, bi * NB, NB) for bi in range(6)]
        + [
            _tile_cols(x_shard, BS - 2 * NB, NB),
            _tile_cols(x_shard, BS - NB, NB),
        ]
    )
    xm = np.stack(
        [_tile_cols(x_shard, 6 * NB + 2 * NB * i, 2 * NB) for i in range(4)]
    )
    xtl = np.zeros((KCH, TGC), X_DT)
    tail = x_shard[:, KC * KCH :].astype(X_DT)  # [BS, 16]
    for bi in range(NBLK):
        q, j = divmod(bi, NGRP)
        xtl[32 * j : 32 * j + KTAIL, q * NB : (q + 1) * NB] = tail[
            bi * NB : (bi + 1) * NB
        ].T
    return (
        np.ascontiguousarray(xa),
        np.ascontiguousarray(xm),
        np.ascontiguousarray(xtl),
    )


def _host_weights(conv_w, w1, b1, w2, b2):
    """Pack all fp16 weights into wpk [128, 906] and biases into bd."""
    # 1/X_SCALE folds into W1e (exact in fp16: pure exponent shift)
    w1e_t = (_fuse_conv_fc1(conv_w, w1).T / X_SCALE).astype(HOST_DT)  # [784, 128]
    w2t = np.asarray(w2, np.float32).T.astype(HOST_DT)  # [128, 10]
    wpk = np.zeros((KCH, 906), HOST_DT)
    wpk[:, 0:768] = (
        w1e_t[0 : KC * KCH].reshape(KC, KCH, HID).transpose(1, 0, 2).reshape(KCH, -1)
    )
    for g in range(NGRP):
        wpk[32 * g : 32 * g + KTAIL, 768:896] = w1e_t[KC * KCH :]
    wpk[:, 896:906] = w2t
    bd = np.zeros((HID, 2), np.float32)
    bd[:, 0] = np.asarray(b1, np.float32)
    for j in range(NGRP):
        bd[32 * j : 32 * j + OUT, 1] = np.asarray(b2, np.float32)
    return np.ascontiguousarray(wpk), np.ascontiguousarray(bd)


def _run(x, conv_w, w1, b1, w2, b2, trace=False):
    x = np.asarray(x, np.float32) * np.float32(X_SCALE)
    wpk, bd = _host_weights(conv_w, w1, b1, w2, b2)

    nc = _build_nc()
    in_maps = []
    for c in range(NCORES):
        xa, xm, xtl = _core_x(x[c * BS : (c + 1) * BS])
        in_maps.append({"xa": xa, "xm": xm, "xtl": xtl, "wpk": wpk, "bd": bd})
    res = run_bass_kernel_spmd(nc, in_maps, list(range(NCORES)), trace=trace)

    y = np.empty((B_TOTAL, OUT), np.float32)
    for c, r in enumerate(res.results):
        # yt[32j+r, 512q+cc] = y[(4q+j)*512+cc, r]
        ytc = np.asarray(r["yt"], np.float32).reshape(NGRP, 32, NGRP, NB)[:, :OUT]
        y[c * BS : (c + 1) * BS] = ytc.transpose(2, 0, 3, 1).reshape(BS, OUT)
    return y, res


def kernel(x, conv_w, w1, b1, w2, b2):
    y, _ = _run(x, conv_w, w1, b1, w2, b2)
    return y


# revision 19
# speedup vs baseline: 1.0078x; 1.0078x over previous
"""Trainium2 Bass kernel for nn_DigitConvolutionalModel (dense_cnn).

Model: y = relu(conv3x3(x) @ w1.T + b1) @ w2.T + b2, x: [65536, 784] f32.

Strategy (v8):
  * Conv3x3 and FC1 fuse on the host into one effective weight
    W1e = w1 @ C with shape [128, 784] (C is the sparse conv operator),
    so the device runs a pure GEMM pipeline:
    y = relu(x @ W1e.T + b1) @ w2.T + b2.
  * Pure data parallel over 8 NeuronCores: each core gets 8192 rows of x.
    No collectives; each core produces its own output shard.
  * x travels as fp8e3 (e3m4), scaled by 2 on the host (absmax 10.8 of
    15.5) with the inverse folded into W1e, which stays fp16 — the PE
    accepts mixed operand dtypes (HW-verified), so weight quantization
    adds no error and x quantization alone costs ~1.3e-2 rel_fro
    (gate: 2e-2). Quarter the HBM traffic of f32 for x; 1 cycle/row on
    the PE; all accumulation stays fp32 in PSUM. DMA delivers each
    512-column block in ~1.1 us vs ~1.7 us of PE work, so the PE never
    starves mid-stream (no HAM clock-gate oscillation).
  * x streams on the SP HWDGE ring only (the ACT ring must stay clear
    for the per-block bias-adds) in a tapered schedule — 6x512,
    4x1024, 2x512 columns — every load one contiguous region
    (128 descriptors) via host pre-tiling, every load with its OWN SBUF
    slot (x fits in SBUF) so nothing waits on buffer recycling. Fine
    granularity up front keeps the PE fed while the stream ramps.
  * Weights / biases load on the ACT HWDGE ring in parallel. The
    16-feature contraction tail (features 768:784, whole batch) is
    packed [128, 2048] across 4 row-groups of 32 partitions so its DMA
    uses all 16 SDMA engines; w1e's tail rows are replicated at
    partition offsets 0/32/64/96 so each block's tail matmul reads its
    group via tile_position.
  * Per 512-column block: 6 accumulating FC1 matmuls + 1 tail matmul
    into a PSUM bank (4-bank rotation), fused bias+ReLU on the vector
    engine (PSUM -> SBUF fp16). The [10, 512] FC2 matmul runs
    SOFTWARE-PIPELINED one block behind FC1 so the PE never waits on
    the relu inside its own stream; its output lands at PSUM partition
    offset 32*(bi%4) (col-group tile_position) so the fp16 output
    accumulator is [128, 2048] — the final store then rides all 16
    SDMA engines (a [10, 8192] layout would ride 2 and add ~6 us).
  * FC2 bias on the scalar engine; ONE final store from the ACT
    sequencer (program order after the last act: no waits).
  * Cross-engine waits are absorbed into the PE stream with tiny dummy
    bf16 ldweights "probes"; the few remaining multi-waits are split via
    event semaphores (bass_rust.generate_event_semaphores).
  * Nine dummy matmuls over a zeroed scratch tile during the DMA-bound
    startup window pre-warm the PE's HAM clock gate to 2.4 GHz.
"""

import os

import ml_dtypes
import numpy as np

import concourse.bass as bass
import concourse.mybir as mybir
import concourse.tile as tile
from concourse.bass import ts
from concourse.bass_utils import run_bass_kernel_spmd

H = W = 28
KH = KW = 3
CIN = H * W  # 784
HID = 128
OUT = 10
B_TOTAL = 65536
NCORES = 8
BS = B_TOTAL // NCORES  # 8192 rows per core
NB = 512  # batch columns per block (fp32 PSUM bank limit)
NBLK = BS // NB  # 16
KCH = 128
KC = 6  # full chunks (6 * 128 = 768)
KTAIL = CIN - KC * KCH  # 16
NGRP = 4  # tail row-groups / FC2 col-groups (32 partitions each)
TGC = BS // NGRP  # tail columns per group (2048)
# tapered x load schedule (columns per load): fine up front
SCHED = [NB] * 6 + [2 * NB] * 4 + [NB] * 2

MM_MODE = os.environ.get("BASS_MM_DT", "f8")
HOST_DT = np.float16
X_DT = ml_dtypes.float8_e3m4
X_SCALE = 2.0  # folded into W1e on the host


def _build_nc():
    f32 = mybir.dt.float32
    mdt = mybir.dt.float16
    xdt = mybir.dt.float8e3
    nc = bass.Bass()
    # x, host-pretiled per load: xa/xm entries are each one contiguous
    # [128, 6, ncols] region (features 0:768)
    xa = nc.dram_tensor("xa", [8, KCH, KC, NB], xdt, kind="ExternalInput")
    xm = nc.dram_tensor("xm", [4, KCH, KC, 2 * NB], xdt, kind="ExternalInput")
    # x contraction tail (features 768:784) packed into 4 row-groups:
    # partition 32g+j = tail feature j of blocks 4g..4g+3
    xtl = nc.dram_tensor("xtl", [KCH, TGC], xdt, kind="ExternalInput")
    # all fp16 weights packed into one tensor -> one DMA:
    # cols 0:768 = w1e chunks [k, c, m]; rows 32g:32g+16 of cols 768:896
    # = the 16-row w1e tail (replicated per row-group g); cols 896:906 =
    # w2t
    wpk = nc.dram_tensor("wpk", [KCH, 906], mdt, kind="ExternalInput")
    # biases in one f32 tensor: col 0 = b1, col 1 rows 32j:32j+10 = b2
    # (replicated per FC2 col-group j)
    bd = nc.dram_tensor("bd", [HID, 2], f32, kind="ExternalInput")
    # output, fp16, col-group packed: yt[32*(bi%4)+r, (bi//4)*512+c] =
    # y[bi*512+c, r]
    yt = nc.dram_tensor("yt", [KCH, NGRP * NB], mdt, kind="ExternalOutput")

    with tile.TileContext(nc) as tc:
        with (
            tc.tile_pool(name="consts", bufs=1) as consts,
            tc.tile_pool(name="xin", bufs=1) as xin,
            tc.tile_pool(name="hpool", bufs=NBLK) as hpool,
            tc.tile_pool(name="ps1", bufs=5, space="PSUM") as ps1p,
            tc.tile_pool(name="ps2", bufs=3, space="PSUM") as ps2p,
        ):
            # All x loads on the SP ring, finest blocks first.
            loads = []
            x_tl = None
            for li, ncols in enumerate(SCHED):
                x_t = xin.tile(
                    [KCH, KC, ncols], xdt, tag=f"x{li}", bufs=1, name=f"x_{li}"
                )
                if li < 6:
                    srcap = xa[li][:]
                elif li < 10:
                    srcap = xm[li - 6][:]
                else:
                    srcap = xa[li - 4][:]
                nc.sync.dma_start(x_t[:], srcap)
                loads.append(x_t)
                if li == 1:
                    # tail after the first two loads: block 0 needs it
                    # only at its 7th matmul (~1.5 us later)
                    x_tl = consts.tile([KCH, TGC], xdt)
                    nc.sync.dma_start(x_tl[:], xtl[:])

            # block bi -> (x tile, column offset)
            def block_src(bi):
                if bi < 6:
                    return loads[bi], 0
                if bi < 14:
                    return loads[6 + (bi - 6) // 2], ((bi - 6) % 2) * NB
                return loads[10 + bi - 14], 0

            # Weights / biases on the ACT ring, in parallel.
            wpk_t = consts.tile([KCH, 906], mdt)
            nc.scalar.dma_start(wpk_t[:], wpk[:])
            w1_t = wpk_t[:, 0:768].rearrange("k (c m) -> k c m", c=KC)
            w2_t = wpk_t[:, 896:906]
            bd_t = consts.tile([HID, 2], f32)
            nc.scalar.dma_start(bd_t[:], bd[:])
            b1_t = bd_t[:, 0:1]

            # fp16 output accumulator [128, 2048]; one store at the end.
            o_all = consts.tile([KCH, NGRP * NB], mdt)

            # Tiny dummy bf16 ldweights "probes" absorb cross-engine
            # waits into the PE's in-order stream ahead of each matmul
            # group (walrus: one sync wait per instruction; the loaded
            # garbage weight is irrelevant, real matmuls self-load).
            def probe(ap, cast=True):
                ap = ap[0:1, 0:1]
                if cast:
                    ap = ap.bitcast(mybir.dt.bfloat16)
                nc.tensor.ldweights(ap)

            # HAM warm-up FIRST: ~8 cold dummy matmuls fill the PE's
            # ~3.4 us activity window back-to-back (no waits), so the
            # clock gate is at 2.4 GHz before the first real matmul.
            # The memset is the first DVE op (ahead of b1_probe, whose
            # wait on the bd DMA would delay it).
            scratch = consts.tile([KCH, NB], mdt)
            nc.vector.memset(scratch[:], 0.0)
            psd = ps2p.tile([HID, NB], f32, tag="warm", bufs=1)

            def dummy_mm():
                nc.tensor.matmul(
                    psd[:], scratch[:, 0:HID], scratch[:], start=True, stop=True
                )

            for _ in range(2):
                dummy_mm()

            # Pre-touch the bias tiles on their consumer engines (b1 on
            # DVE, b2 on ACT) so relu / bias-add need no extra wait.
            b1_probe = consts.tile([1, 1], f32)
            nc.vector.tensor_copy(b1_probe[:], b1_t[0:1, 0:1])
            b2_probe = consts.tile([1, 1], f32)
            nc.scalar.copy(b2_probe[:], bd_t[0:1, 1:2])

            probe(w1_t[:, 0, :])
            probe(x_tl[:], cast=False)
            probe(w2_t[:])

            hs = [None] * NBLK

            def fc2_quad(q):
                """FC2 for quad q (software-pipelined one quad late).
                A tiny N=1 matmul opens the accumulation group (clears
                has_written + pending-zero for the bank); the 4 FC2
                matmuls then target col-groups j=0..3 of that one bank
                (disjoint partitions, start=False = overwrite-where-
                unwritten) and run concurrently in the array; one
                [128, 512] bias-add evacuates the whole quad."""
                ps2 = ps2p.tile([KCH, NB], f32, tag="ps2", bufs=2)
                nc.tensor.matmul(
                    ps2[:, 0:1],
                    scratch[:, 0:HID],
                    scratch[:, 0:1],
                    start=True,
                    stop=False,
                )
                for j in range(NGRP):
                    nc.tensor.matmul(
                        ps2[32 * j : 32 * j + OUT, :],
                        w2_t[:],
                        hs[NGRP * q + j][:],
                        start=False,
                        stop=(j == NGRP - 1),
                        tile_position=(0, 32 * j),
                    )
                nc.scalar.activation(
                    o_all[:, ts(q, NB)],
                    ps2[:],
                    mybir.ActivationFunctionType.Identity,
                    bias=bd_t[:, 1:2],
                )

            for q in range(NGRP):
                pss = []
                # 24 main FC1 matmuls for blocks 4q..4q+3
                for j in range(NGRP):
                    bi = NGRP * q + j
                    x_t, off = block_src(bi)
                    probe(x_t[:, 0, off : off + 1], cast=False)
                    ps = ps1p.tile([HID, NB], f32, tag="ps")
                    for c in range(KC):
                        nc.tensor.matmul(
                            ps[:],
                            w1_t[:, c, :],
                            x_t[:, c, off : off + NB],
                            start=(c == 0),
                            stop=False,
                        )
                    pss.append(ps)
                # 4 tail matmuls on distinct row-groups: concurrent
                for j in range(NGRP):
                    nc.tensor.matmul(
                        pss[j][:],
                        wpk_t[32 * j : 32 * j + KTAIL, 768:896],
                        x_tl[32 * j : 32 * j + KTAIL, ts(q, NB)],
                        start=False,
                        stop=True,
                        tile_position=(32 * j, 0),
                    )
                # relu+bias on DVE: h = max(ps + b1, 0), fp16 out
                for j in range(NGRP):
                    bi = NGRP * q + j
                    h = hpool.tile([HID, NB], mdt, tag="h", name=f"h_{bi}")
                    nc.vector.tensor_scalar(
                        h[:],
                        pss[j][:],
                        b1_t[:],
                        0.0,
                        mybir.AluOpType.add,
                        mybir.AluOpType.max,
                    )
                    hs[bi] = h
                if q >= 1:
                    fc2_quad(q - 1)
                else:
                    dummy_mm()
                    dummy_mm()
            # bulk of the store overlaps the last quad's FC2 work
            nc.scalar.dma_start(
                yt[:, 0 : (NGRP - 1) * NB], o_all[:, 0 : (NGRP - 1) * NB]
            )
            fc2_quad(NGRP - 1)

            # Stores from the ACT sequencer: program order after the acts
            # means they need no cross-engine waits at all.
            nc.scalar.dma_start(
                yt[:, (NGRP - 1) * NB :], o_all[:, (NGRP - 1) * NB :]
            )

    # This walrus build allows one sync-wait per instruction; Tile emits
    # multi-waits in a few places. Split them into event-semaphore
    # chains, same as bacc.compile() does.
    import bass_rust

    bass_rust.generate_event_semaphores(nc)
    return nc


def _fuse_conv_fc1(conv_w, w1):
    """W1e = w1 @ C where C is the 3x3 valid-conv operator [676, 784]."""
    cw = np.asarray(conv_w, np.float64).reshape(KH, KW)
    w1_r = np.asarray(w1, np.float64).reshape(HID, H - KH + 1, W - KW + 1)
    w1e = np.zeros((HID, H, W), np.float64)
    for a in range(KH):
        for b in range(KW):
            w1e[:, a : a + H - KH + 1, b : b + W - KW + 1] += w1_r * cw[a, b]
    return w1e.reshape(HID, CIN).astype(np.float32)


def _tile_cols(x_shard, cs, ncols):
    """[128, 6, ncols] contiguous device layout for columns cs:cs+ncols."""
    return (
        x_shard[cs : cs + ncols, : KC * KCH]
        .reshape(ncols, KC, KCH)
        .transpose(2, 1, 0)
        .astype(X_DT)
    )


def _core_x(x_shard):
    """Pre-tile one core's x rows [BS, 784] into the device layout.
    x arrives pre-scaled by X_SCALE."""
    xa = np.stack(
        [_tile_cols(x_shard, bi * NB, NB) for bi in range(6)]
        + [
            _tile_cols(x_shard, BS - 2 * NB, NB),
            _tile_cols(x_shard, BS - NB, NB),
        ]
    )
    xm = np.stack(
        [_tile_cols(x_shard, 6 * NB + 2 * NB * i, 2 * NB) for i in range(4)]
    )
    xtl = np.zeros((KCH, TGC), X_DT)
    tail = x_shard[:, KC * KCH :].astype(X_DT)  # [BS, 16]
    for bi in range(NBLK):
        q, j = divmod(bi, NGRP)
        xtl[32 * j : 32 * j + KTAIL, q * NB : (q + 1) * NB] = tail[
            bi * NB : (bi + 1) * NB
        ].T
    return (
        np.ascontiguousarray(xa),
        np.ascontiguousarray(xm),
        np.ascontiguousarray(xtl),
    )


def _host_weights(conv_w, w1, b1, w2, b2):
    """Pack all fp16 weights into wpk [128, 906] and biases into bd."""
    # 1/X_SCALE folds into W1e (exact in fp16: pure exponent shift)
    w1e_t = (_fuse_conv_fc1(conv_w, w1).T / X_SCALE).astype(HOST_DT)  # [784, 128]
    w2t = np.asarray(w2, np.float32).T.astype(HOST_DT)  # [128, 10]
    wpk = np.zeros((KCH, 906), HOST_DT)
    wpk[:, 0:768] = (
        w1e_t[0 : KC * KCH].reshape(KC, KCH, HID).transpose(1, 0, 2).reshape(KCH, -1)
    )
    for g in range(NGRP):
        wpk[32 * g : 32 * g + KTAIL, 768:896] = w1e_t[KC * KCH :]
    wpk[:, 896:906] = w2t
    bd = np.zeros((HID, 2), np.float32)
    bd[:, 0] = np.asarray(b1, np.float32)
    for j in range(NGRP):
        bd[32 * j : 32 * j + OUT, 1] = np.asarray(b2, np.float32)
    return np.ascontiguousarray(wpk), np.ascontiguousarray(bd)


def _run(x, conv_w, w1, b1, w2, b2, trace=False):
    x = np.asarray(x, np.float32) * np.float32(X_SCALE)
    wpk, bd = _host_weights(conv_w, w1, b1, w2, b2)

    nc = _build_nc()
    in_maps = []
    for c in range(NCORES):
        xa, xm, xtl = _core_x(x[c * BS : (c + 1) * BS])
        in_maps.append({"xa": xa, "xm": xm, "xtl": xtl, "wpk": wpk, "bd": bd})
    res = run_bass_kernel_spmd(nc, in_maps, list(range(NCORES)), trace=trace)

    y = np.empty((B_TOTAL, OUT), np.float32)
    for c, r in enumerate(res.results):
        # yt[32j+r, 512q+cc] = y[(4q+j)*512+cc, r]
        ytc = np.asarray(r["yt"], np.float32).reshape(NGRP, 32, NGRP, NB)[:, :OUT]
        y[c * BS : (c + 1) * BS] = ytc.transpose(2, 0, 3, 1).reshape(BS, OUT)
    return y, res


def kernel(x, conv_w, w1, b1, w2, b2):
    y, _ = _run(x, conv_w, w1, b1, w2, b2)
    return y


# revision 20
# speedup vs baseline: 1.0140x; 1.0061x over previous
"""Trainium2 Bass kernel for nn_DigitConvolutionalModel (dense_cnn).

Model: y = relu(conv3x3(x) @ w1.T + b1) @ w2.T + b2, x: [65536, 784] f32.

Strategy (v8):
  * Conv3x3 and FC1 fuse on the host into one effective weight
    W1e = w1 @ C with shape [128, 784] (C is the sparse conv operator),
    so the device runs a pure GEMM pipeline:
    y = relu(x @ W1e.T + b1) @ w2.T + b2.
  * Pure data parallel over 8 NeuronCores: each core gets 8192 rows of x.
    No collectives; each core produces its own output shard.
  * x travels as fp8e3 (e3m4), scaled by 2 on the host (absmax 10.8 of
    15.5) with the inverse folded into W1e, which stays fp16 — the PE
    accepts mixed operand dtypes (HW-verified), so weight quantization
    adds no error and x quantization alone costs ~1.3e-2 rel_fro
    (gate: 2e-2). Quarter the HBM traffic of f32 for x; 1 cycle/row on
    the PE; all accumulation stays fp32 in PSUM. DMA delivers each
    512-column block in ~1.1 us vs ~1.7 us of PE work, so the PE never
    starves mid-stream (no HAM clock-gate oscillation).
  * x streams on the SP HWDGE ring only (the ACT ring must stay clear
    for the per-block bias-adds) in a tapered schedule — 6x512,
    4x1024, 2x512 columns — every load one contiguous region
    (128 descriptors) via host pre-tiling, every load with its OWN SBUF
    slot (x fits in SBUF) so nothing waits on buffer recycling. Fine
    granularity up front keeps the PE fed while the stream ramps.
  * Weights / biases load on the ACT HWDGE ring in parallel. The
    16-feature contraction tail (features 768:784, whole batch) is
    packed [128, 2048] across 4 row-groups of 32 partitions so its DMA
    uses all 16 SDMA engines; w1e's tail rows are replicated at
    partition offsets 0/32/64/96 so each block's tail matmul reads its
    group via tile_position.
  * Per 512-column block: 6 accumulating FC1 matmuls + 1 tail matmul
    into a PSUM bank (4-bank rotation), fused bias+ReLU on the vector
    engine (PSUM -> SBUF fp16). The [10, 512] FC2 matmul runs
    SOFTWARE-PIPELINED one block behind FC1 so the PE never waits on
    the relu inside its own stream; its output lands at PSUM partition
    offset 32*(bi%4) (col-group tile_position) so the fp16 output
    accumulator is [128, 2048] — the final store then rides all 16
    SDMA engines (a [10, 8192] layout would ride 2 and add ~6 us).
  * FC2 bias on the scalar engine; ONE final store from the ACT
    sequencer (program order after the last act: no waits).
  * Cross-engine waits are absorbed into the PE stream with tiny dummy
    bf16 ldweights "probes"; the few remaining multi-waits are split via
    event semaphores (bass_rust.generate_event_semaphores).
  * Nine dummy matmuls over a zeroed scratch tile during the DMA-bound
    startup window pre-warm the PE's HAM clock gate to 2.4 GHz.
"""

import os

import ml_dtypes
import numpy as np

import concourse.bass as bass
import concourse.mybir as mybir
import concourse.tile as tile
from concourse.bass import ts
from concourse.bass_utils import run_bass_kernel_spmd

H = W = 28
KH = KW = 3
CIN = H * W  # 784
HID = 128
OUT = 10
B_TOTAL = 65536
NCORES = 8
BS = B_TOTAL // NCORES  # 8192 rows per core
NB = 512  # batch columns per block (fp32 PSUM bank limit)
NBLK = BS // NB  # 16
KCH = 128
KC = 6  # full chunks (6 * 128 = 768)
KTAIL = CIN - KC * KCH  # 16
NGRP = 4  # tail row-groups / FC2 col-groups (32 partitions each)
TGC = BS // NGRP  # tail columns per group (2048)
# tapered x load schedule (columns per load): fine up front
SCHED = [NB] * 6 + [2 * NB] * 4 + [NB] * 2

MM_MODE = os.environ.get("BASS_MM_DT", "f8")
HOST_DT = np.float16
X_DT = ml_dtypes.float8_e3m4
X_SCALE = 2.0  # folded into W1e on the host


def _build_nc():
    f32 = mybir.dt.float32
    mdt = mybir.dt.float16
    xdt = mybir.dt.float8e3
    nc = bass.Bass()
    # x, host-pretiled per load: xa/xm entries are each one contiguous
    # [128, 6, ncols] region (features 0:768)
    xa = nc.dram_tensor("xa", [8, KCH, KC, NB], xdt, kind="ExternalInput")
    xm = nc.dram_tensor("xm", [4, KCH, KC, 2 * NB], xdt, kind="ExternalInput")
    # x contraction tail (features 768:784) packed into 4 row-groups:
    # partition 32g+j = tail feature j of blocks 4g..4g+3
    xtl = nc.dram_tensor("xtl", [KCH, TGC], xdt, kind="ExternalInput")
    # all fp16 weights packed into one tensor -> one DMA:
    # cols 0:768 = w1e chunks [k, c, m]; rows 32g:32g+16 of cols 768:896
    # = the 16-row w1e tail (replicated per row-group g); cols 896:906 =
    # w2t
    wpk = nc.dram_tensor("wpk", [KCH, 906], mdt, kind="ExternalInput")
    # biases in one f32 tensor: col 0 = b1, col 1 rows 32j:32j+10 = b2
    # (replicated per FC2 col-group j)
    bd = nc.dram_tensor("bd", [HID, 2], f32, kind="ExternalInput")
    # output, fp16, col-group packed: yt[32*(bi%4)+r, (bi//4)*512+c] =
    # y[bi*512+c, r]
    yt = nc.dram_tensor("yt", [KCH, NGRP * NB], mdt, kind="ExternalOutput")

    with tile.TileContext(nc) as tc:
        with (
            tc.tile_pool(name="consts", bufs=1) as consts,
            tc.tile_pool(name="xin", bufs=1) as xin,
            tc.tile_pool(name="hpool", bufs=NBLK) as hpool,
            tc.tile_pool(name="ps1", bufs=5, space="PSUM") as ps1p,
            tc.tile_pool(name="ps2", bufs=3, space="PSUM") as ps2p,
        ):
            # All x loads on the SP ring, finest blocks first.
            loads = []
            x_tl = None
            for li, ncols in enumerate(SCHED):
                x_t = xin.tile(
                    [KCH, KC, ncols], xdt, tag=f"x{li}", bufs=1, name=f"x_{li}"
                )
                if li < 6:
                    srcap = xa[li][:]
                elif li < 10:
                    srcap = xm[li - 6][:]
                else:
                    srcap = xa[li - 4][:]
                nc.sync.dma_start(x_t[:], srcap)
                loads.append(x_t)
                if li == 1:
                    # tail after the first two loads: block 0 needs it
                    # only at its 7th matmul (~1.5 us later)
                    x_tl = consts.tile([KCH, TGC], xdt)
                    nc.sync.dma_start(x_tl[:], xtl[:])

            # block bi -> (x tile, column offset)
            def block_src(bi):
                if bi < 6:
                    return loads[bi], 0
                if bi < 14:
                    return loads[6 + (bi - 6) // 2], ((bi - 6) % 2) * NB
                return loads[10 + bi - 14], 0

            # Weights / biases on the ACT ring, in parallel.
            wpk_t = consts.tile([KCH, 906], mdt)
            nc.scalar.dma_start(wpk_t[:], wpk[:])
            w1_t = wpk_t[:, 0:768].rearrange("k (c m) -> k c m", c=KC)
            w2_t = wpk_t[:, 896:906]
            bd_t = consts.tile([HID, 2], f32)
            nc.scalar.dma_start(bd_t[:], bd[:])
            b1_t = bd_t[:, 0:1]

            # fp16 output accumulator [128, 2048]; one store at the end.
            o_all = consts.tile([KCH, NGRP * NB], mdt)

            # Tiny dummy bf16 ldweights "probes" absorb cross-engine
            # waits into the PE's in-order stream ahead of each matmul
            # group (walrus: one sync wait per instruction; the loaded
            # garbage weight is irrelevant, real matmuls self-load).
            def probe(ap, cast=True):
                ap = ap[0:1, 0:1]
                if cast:
                    ap = ap.bitcast(mybir.dt.bfloat16)
                nc.tensor.ldweights(ap)

            # HAM warm-up FIRST: ~8 cold dummy matmuls fill the PE's
            # ~3.4 us activity window back-to-back (no waits), so the
            # clock gate is at 2.4 GHz before the first real matmul.
            # The memset is the first DVE op (ahead of b1_probe, whose
            # wait on the bd DMA would delay it).
            scratch = consts.tile([KCH, NB], mdt)
            nc.vector.memset(scratch[:], 0.0)
            psd = ps2p.tile([HID, NB], f32, tag="warm", bufs=1)

            def dummy_mm():
                nc.tensor.matmul(
                    psd[:], scratch[:, 0:HID], scratch[:], start=True, stop=True
                )

            for _ in range(2):
                dummy_mm()

            # Pre-touch the bias tiles on their consumer engines (b1 on
            # DVE, b2 on ACT) so relu / bias-add need no extra wait.
            b1_probe = consts.tile([1, 1], f32)
            nc.vector.tensor_copy(b1_probe[:], b1_t[0:1, 0:1])
            b2_probe = consts.tile([1, 1], f32)
            nc.scalar.copy(b2_probe[:], bd_t[0:1, 1:2])

            probe(w1_t[:, 0, :])
            probe(x_tl[:], cast=False)
            probe(w2_t[:])

            hs = [None] * NBLK

            def fc2_quad(q):
                """FC2 for quad q (software-pipelined one quad late).
                A tiny N=1 matmul opens the accumulation group (clears
                has_written + pending-zero for the bank); the 4 FC2
                matmuls then target col-groups j=0..3 of that one bank
                (disjoint partitions, start=False = overwrite-where-
                unwritten) and run concurrently in the array; one
                [128, 512] bias-add evacuates the whole quad."""
                ps2 = ps2p.tile([KCH, NB], f32, tag="ps2", bufs=2)
                nc.tensor.matmul(
                    ps2[:, 0:1],
                    scratch[:, 0:HID],
                    scratch[:, 0:1],
                    start=True,
                    stop=False,
                )
                for j in range(NGRP):
                    nc.tensor.matmul(
                        ps2[32 * j : 32 * j + OUT, :],
                        w2_t[:],
                        hs[NGRP * q + j][:],
                        start=False,
                        stop=(j == NGRP - 1),
                        tile_position=(0, 32 * j),
                    )
                nc.scalar.activation(
                    o_all[:, ts(q, NB)],
                    ps2[:],
                    mybir.ActivationFunctionType.Identity,
                    bias=bd_t[:, 1:2],
                )

            for q in range(NGRP):
                pss = []
                # 24 main FC1 matmuls for blocks 4q..4q+3
                for j in range(NGRP):
                    bi = NGRP * q + j
                    x_t, off = block_src(bi)
                    probe(x_t[:, 0, off : off + 1], cast=False)
                    ps = ps1p.tile([HID, NB], f32, tag="ps")
                    for c in range(KC):
                        nc.tensor.matmul(
                            ps[:],
                            w1_t[:, c, :],
                            x_t[:, c, off : off + NB],
                            start=(c == 0),
                            stop=False,
                        )
                    pss.append(ps)
                # 4 tail matmuls on distinct row-groups: concurrent
                for j in range(NGRP):
                    nc.tensor.matmul(
                        pss[j][:],
                        wpk_t[32 * j : 32 * j + KTAIL, 768:896],
                        x_tl[32 * j : 32 * j + KTAIL, ts(q, NB)],
                        start=False,
                        stop=True,
                        tile_position=(32 * j, 0),
                    )
                # relu+bias, fp16 out: h = max(ps + b1, 0) — split
                # across DVE (tensor_scalar) and ACT (activation) so the
                # last quad's four relus don't serialize on one engine
                for j in range(NGRP):
                    bi = NGRP * q + j
                    h = hpool.tile([HID, NB], mdt, tag="h", name=f"h_{bi}")
                    if j % 2 == 0:
                        nc.vector.tensor_scalar(
                            h[:],
                            pss[j][:],
                            b1_t[:],
                            0.0,
                            mybir.AluOpType.add,
                            mybir.AluOpType.max,
                        )
                    else:
                        nc.scalar.activation(
                            h[:],
                            pss[j][:],
                            mybir.ActivationFunctionType.Relu,
                            bias=b1_t[:],
                        )
                    hs[bi] = h
                if q >= 1:
                    fc2_quad(q - 1)
                else:
                    dummy_mm()
                    dummy_mm()
            # bulk of the store overlaps the last quad's FC2 work
            nc.scalar.dma_start(
                yt[:, 0 : (NGRP - 1) * NB], o_all[:, 0 : (NGRP - 1) * NB]
            )
            fc2_quad(NGRP - 1)

            # Stores from the ACT sequencer: program order after the acts
            # means they need no cross-engine waits at all.
            nc.scalar.dma_start(
                yt[:, (NGRP - 1) * NB :], o_all[:, (NGRP - 1) * NB :]
            )

    # This walrus build allows one sync-wait per instruction; Tile emits
    # multi-waits in a few places. Split them into event-semaphore
    # chains, same as bacc.compile() does.
    import bass_rust

    bass_rust.generate_event_semaphores(nc)
    return nc


def _fuse_conv_fc1(conv_w, w1):
    """W1e = w1 @ C where C is the 3x3 valid-conv operator [676, 784]."""
    cw = np.asarray(conv_w, np.float64).reshape(KH, KW)
    w1_r = np.asarray(w1, np.float64).reshape(HID, H - KH + 1, W - KW + 1)
    w1e = np.zeros((HID, H, W), np.float64)
    for a in range(KH):
        for b in range(KW):
            w1e[:, a : a + H - KH + 1, b : b + W - KW + 1] += w1_r * cw[a, b]
    return w1e.reshape(HID, CIN).astype(np.float32)


def _tile_cols(x_shard, cs, ncols):
    """[128, 6, ncols] contiguous device layout for columns cs:cs+ncols."""
    return (
        x_shard[cs : cs + ncols, : KC * KCH]
        .reshape(ncols, KC, KCH)
        .transpose(2, 1, 0)
        .astype(X_DT)
    )


def _core_x(x_shard):
    """Pre-tile one core's x rows [BS, 784] into the device layout.
    x arrives pre-scaled by X_SCALE."""
    xa = np.stack(
        [_tile_cols(x_shard, bi * NB, NB) for bi in range(6)]
        + [
            _tile_cols(x_shard, BS - 2 * NB, NB),
            _tile_cols(x_shard, BS - NB, NB),
        ]
    )
    xm = np.stack(
        [_tile_cols(x_shard, 6 * NB + 2 * NB * i, 2 * NB) for i in range(4)]
    )
    xtl = np.zeros((KCH, TGC), X_DT)
    tail = x_shard[:, KC * KCH :].astype(X_DT)  # [BS, 16]
    for bi in range(NBLK):
        q, j = divmod(bi, NGRP)
        xtl[32 * j : 32 * j + KTAIL, q * NB : (q + 1) * NB] = tail[
            bi * NB : (bi + 1) * NB
        ].T
    return (
        np.ascontiguousarray(xa),
        np.ascontiguousarray(xm),
        np.ascontiguousarray(xtl),
    )


def _host_weights(conv_w, w1, b1, w2, b2):
    """Pack all fp16 weights into wpk [128, 906] and biases into bd."""
    # 1/X_SCALE folds into W1e (exact in fp16: pure exponent shift)
    w1e_t = (_fuse_conv_fc1(conv_w, w1).T / X_SCALE).astype(HOST_DT)  # [784, 128]
    w2t = np.asarray(w2, np.float32).T.astype(HOST_DT)  # [128, 10]
    wpk = np.zeros((KCH, 906), HOST_DT)
    wpk[:, 0:768] = (
        w1e_t[0 : KC * KCH].reshape(KC, KCH, HID).transpose(1, 0, 2).reshape(KCH, -1)
    )
    for g in range(NGRP):
        wpk[32 * g : 32 * g + KTAIL, 768:896] = w1e_t[KC * KCH :]
    wpk[:, 896:906] = w2t
    bd = np.zeros((HID, 2), np.float32)
    bd[:, 0] = np.asarray(b1, np.float32)
    for j in range(NGRP):
        bd[32 * j : 32 * j + OUT, 1] = np.asarray(b2, np.float32)
    return np.ascontiguousarray(wpk), np.ascontiguousarray(bd)


def _run(x, conv_w, w1, b1, w2, b2, trace=False):
    x = np.asarray(x, np.float32) * np.float32(X_SCALE)
    wpk, bd = _host_weights(conv_w, w1, b1, w2, b2)

    nc = _build_nc()
    in_maps = []
    for c in range(NCORES):
        xa, xm, xtl = _core_x(x[c * BS : (c + 1) * BS])
        in_maps.append({"xa": xa, "xm": xm, "xtl": xtl, "wpk": wpk, "bd": bd})
    res = run_bass_kernel_spmd(nc, in_maps, list(range(NCORES)), trace=trace)

    y = np.empty((B_TOTAL, OUT), np.float32)
    for c, r in enumerate(res.results):
        # yt[32j+r, 512q+cc] = y[(4q+j)*512+cc, r]
        ytc = np.asarray(r["yt"], np.float32).reshape(NGRP, 32, NGRP, NB)[:, :OUT]
        y[c * BS : (c + 1) * BS] = ytc.transpose(2, 0, 3, 1).reshape(BS, OUT)
    return y, res


def kernel(x, conv_w, w1, b1, w2, b2):
    y, _ = _run(x, conv_w, w1, b1, w2, b2)
    return y
